# revision 15
# baseline (speedup 1.0000x reference)
"""CondConv3d kernel for 8 TRN2 NeuronCores (v4: warm PE, engine-balanced
batched DMA, dual-ring stores).

Math: the reference einsum 'bi,eocdwh->bocdwh' shares no index between
routing_weights and weight, so it factorizes:
    eff_kernel[b] = (sum_i routing[b,i]) * (sum_e weight[e])
    eff_bias[b]   = (sum_i routing[b,i]) * (sum_e bias[e])
=> out[b] = conv3d(x[b], s_b * W_sum, pad=1) + s_b * bias_sum

Sharding: data-parallel over batch B=8, one sample per core. The
per-sample scalar s_b is folded into that core's weights/bias on host.

Per-core kernel (bf16 in/out, fp32 PSUM accumulate):
  - x in SBUF UNPADDED: [96, 16*4096] bf16; partition bands
    [0,32)/[32,64)/[64,96) hold kd-shifted copies x(d-1)/x(d)/x(d+1).
    The HOST pre-builds the 3 shifted copies partition-linearly in DRAM
    (x3[96, 16, 4096]), so each slot is ONE 768KB 96-partition DMA whose
    outermost DRAM dim is 96 -- SDMA engines are assigned round-robin
    over the outermost DRAM-side AP dim, so this spreads over all 16
    engines (a 3-wide or 4-wide outer dim pins the transfer to 3-4
    engines and serializes the whole kernel; measured).
  - conv = 9 PSUM-accumulated matmuls per 512-output chunk; kh/kw taps
    are free-dim shifts of +-64/+-1, boundary zero-padding realized by
    RESTRICTING the APs (2-D [rows,63] windows). Matmuls interleave the
    4 PE column-tiles (tile_position=(0,32j), one depth slice each) at
    the innermost level: MATMUL starts are pc-monotone, so consecutive
    instructions must target different col-groups to run concurrently
    (same-tile runs of 4 measured 2.1x slower).
  - HAM warm-up: 12 dummy matmuls on a zeroed SBUF block at t=0 so the
    PE clock-gate (1.2->2.4GHz after ~3.4us of activity) opens while
    the first fills are in flight.
  - output: drains (ScalarE/VectorE alternating, fused bias add) into a
    [128,4096] bf16 tile laid out partition-linear; stores go to a
    group-major DRAM tensor o3[4, 128, 4096] (outer dim 128 -> all 16
    engines) from the Scalar HWDGE ring so stores never head-of-line
    block the input fills on the Sync ring. Host un-permutes.
"""

import sys

if "/opt/trn_rl_repo" not in sys.path:
    sys.path.insert(0, "/opt/trn_rl_repo")

import numpy as np
import ml_dtypes

import concourse.bass as bass
import concourse.tile as tile
from concourse import bacc, mybir
from concourse.bass_utils import run_bass_kernel_spmd

# problem shape (hardcoded per contest rules)
B, CI, CO, D, H, W = 8, 32, 32, 16, 64, 64
K = 3
NCORES = 8

SLOT = H * W            # 4096 elements per depth slice per partition
NSTEP = 9
NCHUNK = 8              # 8 chunks of 512 = one 64x64 slice

# tap order: the full-rectangle (kh=1,kw=1) tap goes first so its
# start=True initializes every PSUM position of the chunk.
TAPS = [(1, 1), (1, 0), (1, 2), (0, 0), (0, 1), (0, 2), (2, 0), (2, 1), (2, 2)]

F32 = mybir.dt.float32
BF16 = mybir.dt.bfloat16

WARMUP_MM = 17          # dummy matmuls at t=0 to open the HAM clock gate

_CACHE = {}


def _build_nc():
    # Bacc (vs raw Bass) runs the wait-fixup passes: an ISA instruction can
    # carry only 1 semaphore wait; Bacc spills extras to ldweights/events.
    nc = bacc.Bacc(None)
    x_d = nc.declare_dram_parameter("x", [96, D, SLOT], BF16, isOutput=False)
    w_d = nc.declare_dram_parameter("w", [96, NSTEP * CO], BF16, isOutput=False)
    b_d = nc.declare_dram_parameter("bias", [128, 1], F32, isOutput=False)
    o_d = nc.declare_dram_parameter("out", [4, 128, SLOT], BF16, isOutput=True)

    with tile.TileContext(nc) as tc:
        with (
            tc.tile_pool(name="const", bufs=1) as const,
            tc.tile_pool(name="outs", bufs=3) as outp,
            tc.tile_pool(name="psum", bufs=7, space="PSUM") as psump,
        ):
            xp = const.tile([96, D * SLOT], BF16)
            wsb = const.tile([96, NSTEP, CO], BF16)
            bsb = const.tile([128, 1], F32)
            wz = const.tile([96, 512], BF16)

            nc.scalar.dma_start(
                out=wsb[:, :, :],
                in_=w_d[:].rearrange("p (s o) -> p s o", s=NSTEP),
            )
            nc.scalar.dma_start(out=bsb[:, :], in_=b_d[:])

            # HAM warm-up: the PE clock-gate opens only after ~3.4us of
            # sustained activity; burn that window on zeros while the
            # first x fills are still in flight.
            if WARMUP_MM:
                nc.vector.memset(wz[:, :], 0.0)
                wps = psump.tile([128, 512], F32, name="wps", bufs=1)
                for i in range(WARMUP_MM):
                    nc.tensor.matmul(
                        out=wps[0:32, :],
                        lhsT=wz[0:96, 0:32],
                        rhs=wz[0:96, :],
                        start=True,
                        stop=True,
                        tile_position=(0, 0),
                        skip_group_check=True,
                    )

            def fill(d):
                """One 768KB 96-partition DMA for slot d (outer DRAM dim
                = 96 -> all 16 SDMA engines; per-engine rate is ~16GB/s
                regardless of descriptor size, so slot granularity gives
                the finest completion sems at no bandwidth cost)."""
                nc.sync.dma_start(
                    out=xp[:, d * SLOT : (d + 1) * SLOT], in_=x_d[:, d, :]
                )

            for d in range(D):
                fill(d)

            def compute_group(g, prefetch=()):
                ob = outp.tile([128, SLOT], BF16, name="ob")
                for c8 in range(NCHUNK):
                    # one full PSUM bank = 8 output rows of 64
                    ps = psump.tile([128, 512], F32, name="ps", bufs=7)
                    ps3 = ps[:, :].rearrange("p (h w) -> p h w", h=8)
                    for t, (kh, kw) in enumerate(TAPS):
                        # slice-edge rows whose kh tap would cross the
                        # image boundary are simply not written
                        r0, r1 = 0, 8
                        if kh == 0 and c8 == 0:
                            r0 = 1
                        if kh == 2 and c8 == NCHUNK - 1:
                            r1 = 7
                        for j in range(4):
                            d = 4 * g + j
                            a = d * SLOT + c8 * 512 + (kh - 1) * 64
                            band = slice(32 * j, 32 * j + 32)
                            if kw == 1 and r0 == 0 and r1 == 8:
                                out_ap = ps[band, :]
                                rhs = xp[0:96, a : a + 512]
                            else:
                                rows = r1 - r0
                                ar = a + r0 * 64
                                v3 = xp[
                                    0:96, ar : ar + rows * 64
                                ].rearrange("p (h w) -> p h w", h=rows)
                                if kw == 1:
                                    out_ap = ps3[band, r0:r1, :]
                                    rhs = v3[:, :, :]
                                elif kw == 0:
                                    out_ap = ps3[band, r0:r1, 1:64]
                                    rhs = v3[:, :, 0:63]
                                else:  # kw == 2
                                    out_ap = ps3[band, r0:r1, 0:63]
                                    rhs = v3[:, :, 1:64]
                            nc.tensor.matmul(
                                out=out_ap,
                                lhsT=wsb[0:96, 3 * kh + kw, :],
                                rhs=rhs,
                                start=(t == 0),
                                stop=(t == NSTEP - 1),
                                tile_position=(0, 32 * j),
                                # the 4 col-tiles run disjoint partition
                                # ranges; sim's group tracker is bank-coarse
                                skip_group_check=True,
                            )
                    # drain PSUM -> SBUF bf16 with fused bias add
                    dst3 = ob[:, c8 * 512 : (c8 + 1) * 512]
                    if c8 % 2 == 0:
                        nc.vector.tensor_scalar_add(dst3, ps[:, :], bsb[:, :])
                    else:
                        nc.scalar.activation(
                            out=dst3,
                            in_=ps[:, :],
                            func=mybir.ActivationFunctionType.Identity,
                            bias=bsb[:, :],
                            scale=1.0,
                        )
                    # stream finished chunks out on the Scalar HWDGE ring
                    # (outer DRAM dim = 128 -> all 16 engines); pieces get
                    # finer toward the end so the final transfer is small
                    if g == 3:
                        splits = {3: [2048], 5: [1024], 6: [512], 7: [256, 256]}
                    else:
                        splits = {3: [2048], 7: [2048]}
                    if c8 in splits:
                        hi = (c8 + 1) * 512
                        for step in reversed(splits[c8]):
                            lo = hi - step
                            dst = bass.AP(
                                tensor=o_d,
                                offset=g * 128 * SLOT + lo,
                                ap=[[SLOT, 128], [1, step]],
                            )
                            nc.scalar.dma_start(out=dst, in_=ob[:, lo:hi])
                            hi = lo

            for g in range(4):
                compute_group(g)

    nc.finalize()  # Bacc: runs wait-spill + register allocation passes
    return nc


def _get_nc():
    if "nc" not in _CACHE:
        _CACHE["nc"] = _build_nc()
    return _CACHE["nc"]


def _host_prep(x, routing_weights, weight, bias):
    """Build the per-core input maps (one batch sample per core)."""
    x = np.asarray(x, dtype=np.float32)
    routing_weights = np.asarray(routing_weights, dtype=np.float32)
    weight = np.asarray(weight, dtype=np.float32)
    bias = np.asarray(bias, dtype=np.float32)

    s = routing_weights.sum(axis=1)          # [B]
    w_sum = weight.sum(axis=0)               # [CO, CI, K, K, K]
    b_sum = bias.sum(axis=0)                 # [CO]

    # lhsT layout: [p=(kd,ci), (kh,kw), o]
    wt = np.transpose(w_sum, (2, 1, 3, 4, 0)).reshape(96, NSTEP * CO)

    # partition-linear shifted copies: x3[32g+ci, d, :] = xpad[ci, d+g-1, :]
    xz = np.zeros((B, CI, D + 2, SLOT), dtype=np.float32)
    xz[:, :, 1 : D + 1, :] = x.reshape(B, CI, D, SLOT)
    x3 = np.empty((B, 96, D, SLOT), dtype=np.float32)
    for g in range(3):
        x3[:, 32 * g : 32 * g + 32] = xz[:, :, g : g + D]

    in_maps = []
    for b in range(B):
        wb = (s[b] * wt).astype(ml_dtypes.bfloat16)
        bb = np.tile(s[b] * b_sum, 4).reshape(128, 1).astype(np.float32)
        in_maps.append(
            {
                "x": np.ascontiguousarray(x3[b].astype(ml_dtypes.bfloat16)),
                "w": np.ascontiguousarray(wb),
                "bias": bb,
            }
        )
    return in_maps


def kernel(x, routing_weights, weight, bias):
    in_maps = _host_prep(x, routing_weights, weight, bias)
    nc = _get_nc()
    _CACHE["last_in_maps"] = in_maps
    res = run_bass_kernel_spmd(nc, in_maps, list(range(NCORES)))
    _CACHE["last_result"] = res
    out = np.empty((B, CO, D, H, W), dtype=np.float32)
    for b in range(B):
        r = np.asarray(res.results[b]["out"]).astype(np.float32)
        # o3[g, 32j+o, e] -> out[o, 4g+j, h, w]
        r = r.reshape(4, 4, CO, H, W).transpose(2, 0, 1, 3, 4)
        out[b] = r.reshape(CO, D, H, W)
    return out


# revision 16
# speedup vs baseline: 1.1223x; 1.1223x over previous
"""CondConv3d kernel for 8 TRN2 NeuronCores (v4: warm PE, engine-balanced
batched DMA, dual-ring stores).

Math: the reference einsum 'bi,eocdwh->bocdwh' shares no index between
routing_weights and weight, so it factorizes:
    eff_kernel[b] = (sum_i routing[b,i]) * (sum_e weight[e])
    eff_bias[b]   = (sum_i routing[b,i]) * (sum_e bias[e])
=> out[b] = conv3d(x[b], s_b * W_sum, pad=1) + s_b * bias_sum

Sharding: data-parallel over batch B=8, one sample per core. The
per-sample scalar s_b is folded into that core's weights/bias on host.

Per-core kernel (bf16 in/out, fp32 PSUM accumulate):
  - x in SBUF UNPADDED: [96, 16*4096] bf16; partition bands
    [0,32)/[32,64)/[64,96) hold kd-shifted copies x(d-1)/x(d)/x(d+1).
    The HOST pre-builds the 3 shifted copies partition-linearly in DRAM
    (x3[96, 16, 4096]), so each slot is ONE 768KB 96-partition DMA whose
    outermost DRAM dim is 96 -- SDMA engines are assigned round-robin
    over the outermost DRAM-side AP dim, so this spreads over all 16
    engines (a 3-wide or 4-wide outer dim pins the transfer to 3-4
    engines and serializes the whole kernel; measured).
  - conv = 9 PSUM-accumulated matmuls per 512-output chunk; kh/kw taps
    are free-dim shifts of +-64/+-1, boundary zero-padding realized by
    RESTRICTING the APs (2-D [rows,63] windows). Matmuls interleave the
    4 PE column-tiles (tile_position=(0,32j), one depth slice each) at
    the innermost level: MATMUL starts are pc-monotone, so consecutive
    instructions must target different col-groups to run concurrently
    (same-tile runs of 4 measured 2.1x slower).
  - HAM warm-up: 12 dummy matmuls on a zeroed SBUF block at t=0 so the
    PE clock-gate (1.2->2.4GHz after ~3.4us of activity) opens while
    the first fills are in flight.
  - output: drains (ScalarE/VectorE alternating, fused bias add) into a
    [128,4096] bf16 tile laid out partition-linear; stores go to a
    group-major DRAM tensor o3[4, 128, 4096] (outer dim 128 -> all 16
    engines) from the Scalar HWDGE ring so stores never head-of-line
    block the input fills on the Sync ring. Host un-permutes.
"""

import sys

if "/opt/trn_rl_repo" not in sys.path:
    sys.path.insert(0, "/opt/trn_rl_repo")

import numpy as np
import ml_dtypes

import concourse.bass as bass
import concourse.tile as tile
from concourse import bacc, mybir
from concourse.bass_utils import run_bass_kernel_spmd

# problem shape (hardcoded per contest rules)
B, CI, CO, D, H, W = 8, 32, 32, 16, 64, 64
K = 3
NCORES = 8

SLOT = H * W            # 4096 elements per depth slice per partition
NSTEP = 9
NCHUNK = 8              # 8 chunks of 512 = one 64x64 slice

# tap order: the full-rectangle (kh=1,kw=1) tap goes first so its
# start=True initializes every PSUM position of the chunk.
TAPS = [(1, 1), (1, 0), (1, 2), (0, 0), (0, 1), (0, 2), (2, 0), (2, 1), (2, 2)]

F32 = mybir.dt.float32
BF16 = mybir.dt.bfloat16

WARMUP_MM = 17          # dummy matmuls at t=0 to open the HAM clock gate

_CACHE = {}


def _build_nc():
    # Bacc (vs raw Bass) runs the wait-fixup passes: an ISA instruction can
    # carry only 1 semaphore wait; Bacc spills extras to ldweights/events.
    nc = bacc.Bacc(None)
    x_d = nc.declare_dram_parameter("x", [96, D, SLOT], BF16, isOutput=False)
    w_d = nc.declare_dram_parameter("w", [96, NSTEP * CO], BF16, isOutput=False)
    b_d = nc.declare_dram_parameter("bias", [128, 1], F32, isOutput=False)
    o_d = nc.declare_dram_parameter("out", [4, 128, SLOT], BF16, isOutput=True)

    with tile.TileContext(nc) as tc:
        with (
            tc.tile_pool(name="const", bufs=1) as const,
            tc.tile_pool(name="outs", bufs=3) as outp,
            tc.tile_pool(name="psum", bufs=7, space="PSUM") as psump,
        ):
            xp = const.tile([96, D * SLOT], BF16)
            wsb = const.tile([96, NSTEP, CO], BF16)
            bsb = const.tile([128, 1], F32)
            wz = const.tile([96, 512], BF16)

            nc.scalar.dma_start(
                out=wsb[:, :, :],
                in_=w_d[:].rearrange("p (s o) -> p s o", s=NSTEP),
            )
            nc.scalar.dma_start(out=bsb[:, :], in_=b_d[:])

            # HAM warm-up: the PE clock-gate opens only after ~3.4us of
            # sustained activity; burn that window on zeros while the
            # first x fills are still in flight.
            if WARMUP_MM:
                nc.vector.memset(wz[:, :], 0.0)
                wps = psump.tile([128, 512], F32, name="wps", bufs=1)
                for i in range(WARMUP_MM):
                    nc.tensor.matmul(
                        out=wps[0:32, :],
                        lhsT=wz[0:96, 0:32],
                        rhs=wz[0:96, :],
                        start=True,
                        stop=True,
                        tile_position=(0, 0),
                        skip_group_check=True,
                    )

            def fill(d):
                """One 768KB 96-partition DMA for slot d (outer DRAM dim
                = 96 -> all 16 SDMA engines; per-engine rate is ~16GB/s
                regardless of descriptor size, so slot granularity gives
                the finest completion sems at no bandwidth cost)."""
                nc.sync.dma_start(
                    out=xp[:, d * SLOT : (d + 1) * SLOT], in_=x_d[:, d, :]
                )

            for d in range(4):
                fill(d)

            def compute_group(g, prefetch=()):
                ob = outp.tile([128, SLOT], BF16, name="ob")
                for c8 in range(NCHUNK):
                    if c8 < len(prefetch):
                        prefetch[c8]()
                    # one full PSUM bank = 8 output rows of 64
                    ps = psump.tile([128, 512], F32, name="ps", bufs=7)
                    ps3 = ps[:, :].rearrange("p (h w) -> p h w", h=8)
                    for t, (kh, kw) in enumerate(TAPS):
                        # slice-edge rows whose kh tap would cross the
                        # image boundary are simply not written
                        r0, r1 = 0, 8
                        if kh == 0 and c8 == 0:
                            r0 = 1
                        if kh == 2 and c8 == NCHUNK - 1:
                            r1 = 7
                        for j in range(4):
                            d = 4 * g + j
                            a = d * SLOT + c8 * 512 + (kh - 1) * 64
                            band = slice(32 * j, 32 * j + 32)
                            if kw == 1 and r0 == 0 and r1 == 8:
                                out_ap = ps[band, :]
                                rhs = xp[0:96, a : a + 512]
                            else:
                                rows = r1 - r0
                                ar = a + r0 * 64
                                v3 = xp[
                                    0:96, ar : ar + rows * 64
                                ].rearrange("p (h w) -> p h w", h=rows)
                                if kw == 1:
                                    out_ap = ps3[band, r0:r1, :]
                                    rhs = v3[:, :, :]
                                elif kw == 0:
                                    out_ap = ps3[band, r0:r1, 1:64]
                                    rhs = v3[:, :, 0:63]
                                else:  # kw == 2
                                    out_ap = ps3[band, r0:r1, 0:63]
                                    rhs = v3[:, :, 1:64]
                            nc.tensor.matmul(
                                out=out_ap,
                                lhsT=wsb[0:96, 3 * kh + kw, :],
                                rhs=rhs,
                                start=(t == 0),
                                stop=(t == NSTEP - 1),
                                tile_position=(0, 32 * j),
                                # the 4 col-tiles run disjoint partition
                                # ranges; sim's group tracker is bank-coarse
                                skip_group_check=True,
                            )
                    # drain PSUM -> SBUF bf16 with fused bias add
                    dst3 = ob[:, c8 * 512 : (c8 + 1) * 512]
                    if c8 % 2 == 0:
                        nc.vector.tensor_scalar_add(dst3, ps[:, :], bsb[:, :])
                    else:
                        nc.scalar.activation(
                            out=dst3,
                            in_=ps[:, :],
                            func=mybir.ActivationFunctionType.Identity,
                            bias=bsb[:, :],
                            scale=1.0,
                        )
                    # stream finished chunks out on the Scalar HWDGE ring
                    # (outer DRAM dim = 128 -> all 16 engines); pieces get
                    # finer toward the end so the final transfer is small
                    if g == 3:
                        splits = {3: [2048], 5: [1024], 6: [512], 7: [256, 256]}
                    else:
                        splits = {3: [2048], 7: [2048]}
                    if c8 in splits:
                        hi = (c8 + 1) * 512
                        for step in reversed(splits[c8]):
                            lo = hi - step
                            dst = bass.AP(
                                tensor=o_d,
                                offset=g * 128 * SLOT + lo,
                                ap=[[SLOT, 128], [1, step]],
                            )
                            nc.scalar.dma_start(out=dst, in_=ob[:, lo:hi])
                            hi = lo

            for g in range(4):
                nxt = 4 * (g + 1)
                pre = (
                    [lambda d=nxt + k: fill(d) for k in range(4)]
                    if g + 1 < 4
                    else ()
                )
                compute_group(g, prefetch=pre)

    nc.finalize()  # Bacc: runs wait-spill + register allocation passes
    return nc


def _get_nc():
    if "nc" not in _CACHE:
        _CACHE["nc"] = _build_nc()
    return _CACHE["nc"]


def _host_prep(x, routing_weights, weight, bias):
    """Build the per-core input maps (one batch sample per core)."""
    x = np.asarray(x, dtype=np.float32)
    routing_weights = np.asarray(routing_weights, dtype=np.float32)
    weight = np.asarray(weight, dtype=np.float32)
    bias = np.asarray(bias, dtype=np.float32)

    s = routing_weights.sum(axis=1)          # [B]
    w_sum = weight.sum(axis=0)               # [CO, CI, K, K, K]
    b_sum = bias.sum(axis=0)                 # [CO]

    # lhsT layout: [p=(kd,ci), (kh,kw), o]
    wt = np.transpose(w_sum, (2, 1, 3, 4, 0)).reshape(96, NSTEP * CO)

    # partition-linear shifted copies: x3[32g+ci, d, :] = xpad[ci, d+g-1, :]
    xz = np.zeros((B, CI, D + 2, SLOT), dtype=np.float32)
    xz[:, :, 1 : D + 1, :] = x.reshape(B, CI, D, SLOT)
    x3 = np.empty((B, 96, D, SLOT), dtype=np.float32)
    for g in range(3):
        x3[:, 32 * g : 32 * g + 32] = xz[:, :, g : g + D]

    in_maps = []
    for b in range(B):
        wb = (s[b] * wt).astype(ml_dtypes.bfloat16)
        bb = np.tile(s[b] * b_sum, 4).reshape(128, 1).astype(np.float32)
        in_maps.append(
            {
                "x": np.ascontiguousarray(x3[b].astype(ml_dtypes.bfloat16)),
                "w": np.ascontiguousarray(wb),
                "bias": bb,
            }
        )
    return in_maps


def kernel(x, routing_weights, weight, bias):
    in_maps = _host_prep(x, routing_weights, weight, bias)
    nc = _get_nc()
    _CACHE["last_in_maps"] = in_maps
    res = run_bass_kernel_spmd(nc, in_maps, list(range(NCORES)))
    _CACHE["last_result"] = res
    out = np.empty((B, CO, D, H, W), dtype=np.float32)
    for b in range(B):
        r = np.asarray(res.results[b]["out"]).astype(np.float32)
        # o3[g, 32j+o, e] -> out[o, 4g+j, h, w]
        r = r.reshape(4, 4, CO, H, W).transpose(2, 0, 1, 3, 4)
        out[b] = r.reshape(CO, D, H, W)
    return out
